# revision 26
# baseline (speedup 1.0000x reference)
"""Distributed Trainium2 Bass kernel for a single attention head.

Reference computation (fp32 jax):
    q = queries @ Wq.T + bq        # [B,S,Df]
    k = keys    @ Wk.T + bk
    v = values  @ Wv.T + bv
    attn = softmax((q @ k.T) / sqrt(Df), axis=-1)
    out  = attn @ v                # [B,S,Df]

with B=4, S=4096, D_MODEL=1024, D_FEATURE=64.

Sharding: 8 cores = (batch b in 0..3) x (query-half h in 0..1).
Core c handles batch b=c//2, q rows [h*2048, (h+1)*2048). Each core gets
its q-half plus the FULL keys/values of its batch (no collectives).

v5 design notes (on top of v4, see git history / kernel_v4_baseline).
Machine constants: ACT exp on [128,1024] psum->sbuf costs ~1336ns
((~579+N)/1.2); 64 of them = 85.5us, the kernel floor. A 512-col
matmul issues every ~259ns warm; disjoint row/col-group matmuls run
concurrently. ~7us NEFF preamble before the first kernel op; each
dma_start costs ~730ns on its issuing queue.
  - DMA issues are spread across engines: sync does wqk+q blocks,
    gpsimd does k/v blocks, vector does bias. xk is laid out in
    256-col sub-blocks so the first exp waits on only ~1MB
    (wqk + q block0 + k sub0); the first pair's exps are [128,512].
  - v-projection is column-tiled: two 512-col blocks project
    concurrently in the two column halves of the PE array (out
    partitions 0:64 / 64:128 via tile_position), halving its PE
    stream time. Pairs run at kb1/3/5 (blocks 1+2, 3+4, 5+6);
    blocks 0 and 7 project solo at kb0/kb6.
  - tail: the filler drain stops 4 thunks short (G4 jc30/31), so
    merge_A + finals_A run under the last B-half exps; then the last
    G4 matmuls, merge_B, finals_B. finals batch 8 transposes into one
    psum strip, one strided reciprocal, 8 muls.
"""

import numpy as np
import ml_dtypes

import concourse.bass as bass
import concourse.mybir as mybir
import concourse.tile as tile
from concourse import bacc
from concourse.bass_utils import run_bass_kernel_spmd
from concourse.masks import make_identity

B = 4
S = 4096
DM = 1024
DF = 64
NCORES = 8
SQ = S // 2          # local q rows per core
MC = DM // 128       # 8 contraction chunks
MCP = MC // 2        # 4 DoubleRow chunk-pairs
NI = 512             # moving-operand tile (one PSUM bank of fp32)
NI2 = 256            # xk sub-block width
JC = S // 128        # 32 key chunks
NBQ = SQ // NI       # 4 q column blocks
NBK = S // NI        # 8 k/v column blocks
IP = SQ // 2         # 1024: i-rows per attention half
QW = MCP * 2 * NI    # 4096 fp8 elements per q block
QW2 = MCP * 2 * NI2  # 2048 fp8 elements per k sub-block
BF16 = mybir.dt.bfloat16
F8 = mybir.dt.float8e4
F32 = mybir.dt.float32
NP_BF16 = ml_dtypes.bfloat16
NP_F8 = ml_dtypes.float8_e4m3
EXP = mybir.ActivationFunctionType.Exp

# parked exp outputs: all 32 B-half chunks + A-half chunks 16..23
NPARK = JC + 8

# debug hook: when non-empty, build_kernel emits extra dram taps
DBG = {}

# vT2 layout: per 512-col block -> (partition base, column offset).
# Paired blocks share a column range, one in each partition half.
VMAP = {0: (0, 0), 1: (64, 0), 2: (0, 512), 3: (64, 512),
        4: (0, 1024), 5: (64, 1024), 6: (0, 1536), 7: (64, 1536)}


def _park_idx(jc, ipass):
    if ipass == 1:
        return jc
    assert 16 <= jc < 24
    return JC + (jc - 16)


def build_kernel(tc):
    nc = tc.nc
    xq = nc.dram_tensor("xq", [128, NBQ * QW], F8, kind="ExternalInput")
    xk = nc.dram_tensor("xk", [128, 2 * NBK * QW2], F8, kind="ExternalInput")
    xv = nc.dram_tensor("xv", [128, NBK * MC * NI], BF16, kind="ExternalInput")
    wqk = nc.dram_tensor("wqk", [128, 2 * MCP * 2 * 128], F8, kind="ExternalInput")
    wv = nc.dram_tensor("wv", [128, MC * DF], BF16, kind="ExternalInput")
    bias = nc.dram_tensor("bias", [128, 3], F32, kind="ExternalInput")
    out = nc.dram_tensor("out", [128, 2 * (IP // 128) * DF], F32,
                         kind="ExternalOutput")

    from contextlib import ExitStack

    with ExitStack() as ctx:
        const_pool = ctx.enter_context(tc.tile_pool(name="const", bufs=1))
        xq_pool = ctx.enter_context(tc.tile_pool(name="xq", bufs=4))
        xk_pool = ctx.enter_context(tc.tile_pool(name="xk", bufs=4))
        xv_pool = ctx.enter_context(tc.tile_pool(name="xv", bufs=4))
        act_pool = ctx.enter_context(tc.tile_pool(name="act", bufs=1))
        pt_pool = ctx.enter_context(tc.tile_pool(name="pt", bufs=8))
        outT_pool = ctx.enter_context(tc.tile_pool(name="outT", bufs=2))
        fin_pool = ctx.enter_context(tc.tile_pool(name="fin", bufs=2))
        # PSUM budget (8 banks): ppsum 2x[128,512] = 2 banks (proj psum
        # + vtrans scratch, then the G4 accumulator pair), spsum
        # 2x[128,1024] = 4 banks (scores double-buffer, then finals_B
        # scratch), opsum [65,1024] = 2 banks (G1/G2/G3 in sequence,
        # then finals_A scratch).
        ppsum = ctx.enter_context(tc.tile_pool(name="ppsum", bufs=2, space="PSUM"))
        spsum = ctx.enter_context(tc.tile_pool(name="spsum", bufs=2, space="PSUM"))
        opsum = ctx.enter_context(tc.tile_pool(name="opsum", bufs=1, space="PSUM"))

        # ---- DMAs, all on sync: only sync/scalar (hwdge engines)
        # stripe a dma_start across the 16 hardware queues; gpsimd
        # issues run on a single queue at ~1/10 the bandwidth. Issue
        # order = deadline order; the first exp needs only
        # wqk + q block0 + k sub-block0 (~1MB). bias rides scalar
        # (tiny, and scalar is idle before the first exp). ----
        wqk_sb = const_pool.tile([128, 2 * MCP * 2 * 128], F8, tag="wqk")
        nc.sync.dma_start(wqk_sb[:], wqk[:])
        bias_sb = const_pool.tile([128, 3], F32, tag="bias")
        nc.scalar.dma_start(bias_sb[:], bias[:])
        wv_sb = const_pool.tile([128, MC * DF], BF16, tag="wv")
        q_tiles = [None] * NBQ

        def load_q(i):
            t = xq_pool.tile([128, QW], F8, tag="xq")
            nc.sync.dma_start(t[:], xq[:, i * QW:(i + 1) * QW])
            q_tiles[i] = t

        k_tiles = {}
        v_tiles = {}

        def load_k0():
            # first k block in two sub-block dma_starts so kproj can
            # start after 256KB
            t = xk_pool.tile([128, 2 * QW2], F8, tag="xk")
            nc.sync.dma_start(t[:, 0:QW2], xk[:, 0:QW2])
            nc.sync.dma_start(t[:, QW2:2 * QW2], xk[:, QW2:2 * QW2])
            k_tiles[0] = t

        def load_k(i):
            t = xk_pool.tile([128, 2 * QW2], F8, tag="xk")
            nc.sync.dma_start(t[:], xk[:, i * 2 * QW2:(i + 1) * 2 * QW2])
            k_tiles[i] = t

        def load_v(i):
            t = xv_pool.tile([128, MC * NI], BF16, tag="xv")
            nc.sync.dma_start(t[:], xv[:, i * MC * NI:(i + 1) * MC * NI])
            v_tiles[i] = t

        load_q(0)
        load_k0()
        load_q(1)
        load_q(2)
        load_q(3)
        load_v(0)
        load_k(1)
        load_v(1)
        nc.sync.dma_start(wv_sb[:], wv[:])
        load_k(2)
        load_v(2)
        load_k(3)
        load_v(3)
        load_k(4)
        load_v(4)
        load_k(5)
        load_v(5)
        load_k(6)
        load_v(6)
        load_k(7)
        load_v(7)

        def q_ap(i):
            return q_tiles[i][:]

        # ---- constants computed on-chip ----
        scratch = const_pool.tile([DF, 1], F32, tag="scratch")
        ident = const_pool.tile([128, 128], BF16, tag="ident")
        make_identity(nc, ident[:])
        identf = const_pool.tile([128, 128], F32, tag="identf")
        make_identity(nc, identf[:])
        # preload the ACT exp table while DMAs stream
        nc.scalar.activation(scratch[:], identf[0:DF, 0:1], EXP)

        # ---- PE warm-up: dummy matmuls cover the HAM ramp until the
        # first q-projection's input DMA lands ----
        warm = opsum.tile([DF, 128], F32, tag="po")
        for _ in range(50):
            nc.tensor.matmul(warm[:], ident[:, 0:DF], ident[:], start=True, stop=True)

        # ---- persistent activations ----
        qT_sb = act_pool.tile([128, SQ], BF16, tag="qT")
        kT_sb = act_pool.tile([128, S], BF16, tag="kT")
        vT2_sb = act_pool.tile([128, 4 * NI], BF16, tag="vT2")
        v_sb = act_pool.tile([128, JC * (DF + 1)], BF16, tag="v")  # [128, 32*65]
        nc.gpsimd.memset(v_sb[:], 1.0)  # col DF of every block stays 1.0
        park_sb = act_pool.tile([128, NPARK * IP], BF16, tag="park")  # 10 MB

        def project_block_q(x_ap, i):
            """One 512-col q projection block: 4 fp8 DoubleRow matmuls
            with [w|w]-duplicated stationaries so the result lands in
            both partition halves (score pairing reads them as
            independent 64-row PE tiles)."""
            ps = ppsum.tile([128, NI], F32, tag="ps")
            for p in range(MCP):
                o = p * 256
                nc.tensor.matmul(
                    ps[:],
                    wqk_sb[:, o:o + 256].rearrange("p (s c) -> p s c", s=2),
                    x_ap[:, p * 2 * NI:(p + 1) * 2 * NI].rearrange(
                        "p (s c) -> p s c", s=2),
                    start=(p == 0), stop=(p == MCP - 1),
                    perf_mode=mybir.MatmulPerfMode.DoubleRow,
                )
            nc.vector.tensor_scalar_add(
                qT_sb[:, i * NI:(i + 1) * NI], ps[:],
                bias_sb[0:128, 0:1])

        def project_block_k(i):
            """One 512-col k projection block as two 256-col halves so
            the first scores can start after half the block's DMA."""
            t = k_tiles[i]
            ps = ppsum.tile([128, NI], F32, tag="ps")
            for sub in range(2):
                for p in range(MCP):
                    o = MCP * 2 * 128 + p * 256
                    nc.tensor.matmul(
                        ps[:, sub * NI2:(sub + 1) * NI2],
                        wqk_sb[:, o:o + 256].rearrange("p (s c) -> p s c", s=2),
                        t[:, sub * QW2 + p * 2 * NI2:
                          sub * QW2 + (p + 1) * 2 * NI2].rearrange(
                            "p (s c) -> p s c", s=2),
                        start=(p == 0), stop=(p == MCP - 1),
                        perf_mode=mybir.MatmulPerfMode.DoubleRow,
                    )
                nc.vector.tensor_scalar_add(
                    kT_sb[:, i * NI + sub * NI2:i * NI + (sub + 1) * NI2],
                    ps[:, sub * NI2:(sub + 1) * NI2],
                    bias_sb[0:128, 1:2])

        def project_v_pair(bA, bB, mcs, evict):
            """Part of a column-tiled v projection block pair: block bA
            lands in psum partitions 0:64 (array col groups 0-1), bB in
            64:128 (groups 2-3); the two matmuls of each m-chunk run
            concurrently. Separate banks so the two accumulation
            groups don't share a psum zero region."""
            pbA, coA = VMAP[bA]
            pbB, coB = VMAP[bB]
            assert pbA == 0 and pbB == 64 and coA == coB
            if 0 in mcs:
                project_v_pair.psA = ppsum.tile([DF, NI], F32, tag="ps")
                project_v_pair.psB = ppsum.tile([128, NI], F32, tag="ps")
            psA = project_v_pair.psA
            psB = project_v_pair.psB
            for mc_i in mcs:
                w_ap = wv_sb[:, mc_i * DF:(mc_i + 1) * DF]
                nc.tensor.matmul(
                    psA[:], w_ap,
                    v_tiles[bA][:, mc_i * NI:(mc_i + 1) * NI],
                    start=(mc_i == 0), stop=(mc_i == MC - 1),
                    tile_position=(0, 0),
                )
                nc.tensor.matmul(
                    psB[DF:128, :], w_ap,
                    v_tiles[bB][:, mc_i * NI:(mc_i + 1) * NI],
                    start=(mc_i == 0), stop=(mc_i == MC - 1),
                    tile_position=(0, 64),
                )
            if evict:
                nc.vector.tensor_scalar_add(
                    vT2_sb[0:DF, coA:coA + NI], psA[:], bias_sb[0:DF, 2:3])
                nc.vector.tensor_scalar_add(
                    vT2_sb[DF:128, coA:coA + NI], psB[DF:128, :],
                    bias_sb[DF:128, 2:3])

        def vtrans(jc):
            b = jc // 4
            pb, co = VMAP[b]
            pv = ppsum.tile([128, DF], BF16, tag="ps")
            src = vT2_sb[pb:pb + DF, co + (jc % 4) * 128:co + (jc % 4) * 128 + 128]
            nc.tensor.transpose(pv[:], src, ident[pb:pb + DF, pb:pb + DF])
            nc.vector.tensor_copy(
                v_sb[:, jc * (DF + 1):jc * (DF + 1) + DF], pv[:])

        def v_slice(jc):
            return v_sb[:, jc * (DF + 1):jc * (DF + 1) + DF + 1]

        # ---- thunk queue: every attn@v matmul (and accumulator
        # rotation step) is drip-fed between exp groups. push_buf lags
        # exp-adjacent thunks one pair so popped matmuls never stall
        # the PE FIFO on a not-yet-run exp. ----
        filler_q = []
        push_buf = []

        def pop_fillers(n=8):
            for _ in range(min(n, len(filler_q))):
                filler_q.pop(0)()

        poG = {}
        outT_A = outT_pool.tile([DF + 1, IP], F32, tag="outT", name="outT_A")
        outT_B = outT_pool.tile([DF + 1, IP], F32, tag="outT", name="outT_B")

        def attnv_mm(gen, jc, src_ap, ii, first, last):
            po = poG[gen]
            if gen == "G4":
                nc.tensor.matmul(
                    po[ii][:], v_slice(jc), src_ap[:, ii * NI:(ii + 1) * NI],
                    start=first, stop=last)
            else:
                nc.tensor.matmul(
                    po[:, ii * NI:(ii + 1) * NI], v_slice(jc),
                    src_ap[:, ii * NI:(ii + 1) * NI],
                    start=first, stop=last)

        def push_mms(buf, gen, jc, src, first, last):
            for ii in range(2):
                buf.append(lambda gen=gen, jc=jc, src=src, ii=ii:
                           attnv_mm(gen, jc, src, ii, first, last))

        def park_ap(jc, ipass):
            idx = _park_idx(jc, ipass)
            return park_sb[:, idx * IP:(idx + 1) * IP]

        def hooks(jc, ipass, pts):
            """Queue the attn@v matmuls for one exp'd chunk."""
            if ipass == 1:
                if jc >= 28:
                    push_mms(push_buf, "G4", jc, pts, False, jc == 31)
                return
            if jc < 16:
                push_mms(push_buf, "G1", jc, pts, jc == 0, jc == 15)
            elif jc >= 24:
                push_mms(push_buf, "G3", jc, pts, False, jc == 31)

        def attn_pair(jc0, ipasses=(0, 1), mid=None, pop_n=8):
            """Scores + exp for the requested i-halves of chunks jc0,
            jc0+1. Per half: scores, exps, `mid` (once), fillers; hook
            thunks lag one attn_pair via push_buf."""
            filler_q.extend(push_buf)
            push_buf.clear()
            pending = None
            for ipass in ipasses:
                io = ipass * IP
                ss0 = spsum.tile([128, IP], F32, tag="ss", name="ss0")
                ss1 = spsum.tile([128, IP], F32, tag="ss", name="ss1")
                sss = [ss0, ss1]
                for ii in range(IP // NI):
                    for t in range(2):
                        jc = jc0 + t
                        p0 = t * DF
                        nc.tensor.matmul(
                            sss[t][:, ii * NI:(ii + 1) * NI],
                            kT_sb[p0:p0 + DF, jc * 128:(jc + 1) * 128],
                            qT_sb[p0:p0 + DF, io + ii * NI:io + (ii + 1) * NI],
                            start=True, stop=True,
                        )
                if pending:
                    for args in pending:
                        hooks(*args)
                    pending = None
                group = []
                for t in range(2):
                    jc = jc0 + t
                    parked = (ipass == 1) or (16 <= jc < 24)
                    if parked:
                        pts = park_ap(jc, ipass)
                    else:
                        pts = pt_pool.tile([128, IP], BF16, tag="pt")
                    nc.scalar.activation(pts[:], sss[t][:], EXP, scale=0.125)
                    group.append((jc, ipass, pts))
                if mid is not None:
                    mid()
                    mid = None
                pop_fillers(pop_n)
                pending = group
            for args in pending:
                hooks(*args)

        # ================= kb0: get ACT started ASAP =================
        # pair(0) is hand-rolled with [128,512] exps so the first exp
        # waits only on wqk + q block0 + k sub-block0 (~1MB of DMA).
        poG["G1"] = opsum.tile([DF + 1, IP], F32, tag="po", name="poG1")
        project_block_q(q_ap(0), 0)
        project_block_k(0)
        ss0 = spsum.tile([128, IP], F32, tag="ss", name="ss0")
        ss1 = spsum.tile([128, IP], F32, tag="ss", name="ss1")
        pts0 = pt_pool.tile([128, IP], BF16, tag="pt")
        pts1 = pt_pool.tile([128, IP], BF16, tag="pt")
        for ii in range(2):
            for t in range(2):
                ss = [ss0, ss1][t]
                pts = [pts0, pts1][t]
                p0 = t * DF
                nc.tensor.matmul(
                    ss[:, ii * NI:(ii + 1) * NI],
                    kT_sb[p0:p0 + DF, t * 128:(t + 1) * 128],
                    qT_sb[p0:p0 + DF, ii * NI:(ii + 1) * NI],
                    start=True, stop=True,
                )
                nc.scalar.activation(pts[:, ii * NI:(ii + 1) * NI],
                                     ss[:, ii * NI:(ii + 1) * NI],
                                     EXP, scale=0.125)
            if ii == 0:
                project_block_q(q_ap(1), 1)
        hooks(0, 0, pts0)
        hooks(1, 0, pts1)
        # q blocks 2,3 feed the deferred B halves starting at def(0)
        project_block_q(q_ap(2), 2)
        project_block_q(q_ap(3), 3)

        def kb0_mid():
            project_block_k(1)
            project_v_pair(0, 1, range(MC), evict=True)
            for jc in range(4):
                vtrans(jc)

        # B halves of chunks 0..15 are deferred filler ACT work: two
        # deferred pairs per kb at kb0..kb3 give every kb exactly 8
        # exp tiles, and their emission before each kb's projection
        # mountain keeps ACT fed while PE grinds. pair(2)'s pops are
        # held (pop_n=0) so the G1 jc0/1 matmuls queue after
        # kb0_mid's vtrans.
        deferred_b = [0, 2, 4, 6, 8, 10, 12, 14]
        attn_pair(2, ipasses=(0,), pop_n=0)
        attn_pair(deferred_b.pop(0), ipasses=(1,), mid=kb0_mid)
        attn_pair(deferred_b.pop(0), ipasses=(1,))

        # v-projection plan: column-tiled pairs at even kbs (blocks
        # projected one kb before their chunks are consumed; pair
        # (0,1) in kb0's mid); vtrans for a block runs in the kb that
        # consumes it.
        VPAIR = {2: (2, 3), 4: (4, 5), 6: (6, 7)}

        # ================= kb loop 1..7 =================
        for kb in range(1, NBK):
            # flush lagged thunks so kb-head rotation steps queue after
            # the previous pair's attn@v matmuls
            filler_q.extend(push_buf)
            push_buf.clear()

            def mid1(kb=kb):
                if kb in VPAIR:
                    project_v_pair(*VPAIR[kb], mcs=range(4), evict=False)
                elif kb == 7:
                    # all four vtrans in pair1 so the G4 ppsum
                    # allocation (popped after the mid) comes last in
                    # the ppsum rotation
                    for jc in range(4 * kb, 4 * kb + 4):
                        vtrans(jc)
                else:
                    for jc in range(4 * kb, 4 * kb + 2):
                        vtrans(jc)

            def mid2(kb=kb):
                if kb in VPAIR:
                    project_v_pair(*VPAIR[kb], mcs=range(4, MC), evict=True)
                if kb < 7:
                    project_block_k(kb + 1)
                if kb in VPAIR:
                    for jc in range(4 * kb, 4 * kb + 4):
                        vtrans(jc)
                elif kb == 7:
                    pass
                else:
                    for jc in range(4 * kb + 2, 4 * kb + 4):
                        vtrans(jc)

            pop_n = 12 if kb >= 6 else (10 if kb >= 4 else 8)

            if kb == 4:
                # rotate the opsum slot G1 -> G2 in queue order, after
                # G1's lagged jc15 matmuls
                def rot_g2():
                    nc.vector.tensor_copy(outT_A[:], poG["G1"][:])
                    if DBG.get("taps"):
                        g1tap = nc.dram_tensor("tap_g1", [DF + 1, IP], F32,
                                               kind="ExternalOutput")
                        nc.sync.dma_start(g1tap[:], outT_A[:])
                    poG["G2"] = opsum.tile([DF + 1, IP], F32, tag="po",
                                           name="poG2")
                filler_q.append(rot_g2)
                for jc in range(0, 16):
                    push_mms(filler_q, "G2", jc, park_ap(jc, 1),
                             jc == 0, jc == 15)
            if kb == 6:
                def rot_g3():
                    nc.vector.tensor_copy(outT_B[:], poG["G2"][:])
                    poG["G3"] = opsum.tile([DF + 1, IP], F32, tag="po",
                                           name="poG3")
                filler_q.append(rot_g3)
                for jc in range(16, 24):
                    push_mms(filler_q, "G3", jc, park_ap(jc, 0),
                             jc == 16, False)
            if kb == 7:
                # after kb6's last vtrans, the ppsum banks become the
                # two G4 accumulator halves
                def alloc_g4():
                    g4a = ppsum.tile([DF + 1, NI], F32, tag="ps", name="g4a")
                    g4b = ppsum.tile([DF + 1, NI], F32, tag="ps", name="g4b")
                    poG["G4"] = [g4a, g4b]
                filler_q.append(alloc_g4)
                for jc in range(16, 28):
                    push_mms(filler_q, "G4", jc, park_ap(jc, 1),
                             jc == 16, False)

            if kb in (1, 2, 3):
                attn_pair(deferred_b.pop(0), ipasses=(1,), pop_n=pop_n)
                attn_pair(deferred_b.pop(0), ipasses=(1,), pop_n=pop_n)
            ip = (0,) if kb <= 3 else (0, 1)
            attn_pair(4 * kb, ipasses=ip, mid=mid1, pop_n=pop_n)
            attn_pair(4 * kb + 2, ipasses=ip, mid=mid2, pop_n=pop_n)

        assert not deferred_b

        # ---- finals: 8 transposes into one psum strip, one strided
        # reciprocal over the denominator columns, 8 muls, one DMA ----
        def finals(outT_sb, ipass, pool, tag):
            # 8 transposes into one 2-bank psum strip (128-col slots so
            # no [128,65] output crosses a bank), one strided
            # reciprocal over the 8 denominator columns, 8 muls.
            ob = fin_pool.tile([128, (IP // 128) * DF], F32, tag="ob")
            pf = pool.tile([128, IP], F32, tag=tag, name=f"pf{ipass}")
            for c in range(IP // 128):
                nc.tensor.transpose(
                    pf[:, c * 128:c * 128 + DF + 1],
                    outT_sb[:, c * 128:(c + 1) * 128],
                    identf[0:DF + 1, 0:DF + 1])
            rcp = fin_pool.tile([128, IP // 128], F32, tag="rcp")
            nc.vector.reciprocal(
                rcp[:], pf[:].rearrange("p (c k) -> p c k", k=128)[:, :, DF:DF + 1])
            nc.vector.tensor_tensor(
                ob[:].rearrange("p (c k) -> p c k", k=DF),
                pf[:].rearrange("p (c k) -> p c k", k=128)[:, :, 0:DF],
                rcp[:].rearrange("p (c k) -> p c k", k=1).broadcast_to(
                    [128, IP // 128, DF]),
                op=mybir.AluOpType.mult)
            half = (IP // 128) * DF
            nc.sync.dma_start(out[:, ipass * half:(ipass + 1) * half], ob[:])

        # ---- tail: drain all thunks except the last G4 pair (jc30/31,
        # gated on the final B exps), so merge_A + finals_A overlap the
        # last exps; then the G4 tail, merge_B, finals_B. ----
        filler_q.extend(push_buf)
        push_buf.clear()
        assert len(filler_q) >= 4
        while len(filler_q) > 4:
            filler_q.pop(0)()
        if DBG.get("taps"):
            g3tap = nc.dram_tensor("tap_g3", [DF + 1, IP], F32,
                                   kind="ExternalOutput")
            fin3 = fin_pool.tile([DF + 1, IP], F32, tag="ob", name="fin3")
            nc.vector.tensor_copy(fin3[:], poG["G3"][:])
            nc.sync.dma_start(g3tap[:], fin3[:])
        nc.vector.tensor_add(outT_A[:], outT_A[:], poG["G3"][:])
        finals(outT_A, 0, opsum, "po")
        while filler_q:
            filler_q.pop(0)()
        nc.vector.tensor_add(outT_B[:, 0:NI], outT_B[:, 0:NI], poG["G4"][0][:])
        nc.vector.tensor_add(outT_B[:, NI:IP], outT_B[:, NI:IP], poG["G4"][1][:])
        finals(outT_B, 1, spsum, "ss")


_COMPILED = None


def get_compiled():
    global _COMPILED
    if _COMPILED is None:
        nc = bacc.Bacc("TRN2", target_bir_lowering=False, debug=False,
                       enable_asserts=False, num_devices=NCORES)
        with tile.TileContext(nc) as tc:
            build_kernel(tc)
        nc.compile()
        _COMPILED = nc
    return _COMPILED


def _to_pair_major(xT, ni):
    """[DM, s_len] fp32 -> fp8 [128, nblk * 4 pairs * 2 slots * ni]."""
    s_len = xT.shape[1]
    nblk = s_len // ni
    r = xT.reshape(MCP, 2, 128, nblk, ni).transpose(2, 3, 0, 1, 4)
    return np.ascontiguousarray(r.reshape(128, nblk * MCP * 2 * ni)).astype(NP_F8)


def _w_pair_major(W):
    """W [64, DM] fp32 -> fp8 [128, 4 pairs * 2 slots * 128] with [w|w] dup."""
    WT = np.ascontiguousarray(np.asarray(W, np.float32).T)   # [DM, 64]
    dup = np.concatenate([WT, WT], axis=1)                   # [DM, 128]
    r = dup.reshape(MCP, 2, 128, 128).transpose(2, 0, 1, 3)
    return np.ascontiguousarray(r.reshape(128, MCP * 2 * 128)).astype(NP_F8)


def _to_block_major(xT):
    """[DM, s_len] -> bf16 [128, nblk*MC*NI]: 512-col blocks, m-chunk-major."""
    s_len = xT.shape[1]
    nblk = s_len // NI
    return np.ascontiguousarray(
        xT.reshape(MC, 128, nblk, NI).transpose(1, 2, 0, 3)
        .reshape(128, nblk * MC * NI)).astype(NP_BF16)


def _w_chunk_major(W, dt):
    """W [64, DM] -> [128, MC*64]: per m-chunk [128, 64] stationaries."""
    WT = np.ascontiguousarray(np.asarray(W, np.float32).T)   # [DM, 64]
    return np.ascontiguousarray(
        WT.reshape(MC, 128, DF).transpose(1, 0, 2).reshape(128, MC * DF)
    ).astype(dt)


def make_in_maps(queries, keys, values, Wq, bq, Wk, bk, Wv, bv):
    queries = np.asarray(queries, dtype=np.float32)
    keys = np.asarray(keys, dtype=np.float32)
    values = np.asarray(values, dtype=np.float32)
    wqk_host = np.concatenate([_w_pair_major(Wq), _w_pair_major(Wk)], axis=1)
    wv_host = _w_chunk_major(Wv, NP_BF16)
    bias64 = np.stack(
        [np.asarray(bq), np.asarray(bk), np.asarray(bv)], axis=1
    ).astype(np.float32)
    bias_host = np.concatenate([bias64, bias64], axis=0)     # [128, 3]

    in_maps = []
    for c in range(NCORES):
        b, h = c // 2, c % 2
        in_maps.append({
            "xq": _to_pair_major(queries[b, h * SQ:(h + 1) * SQ, :].T, NI),
            "xk": _to_pair_major(keys[b].T, NI2),
            "xv": _to_block_major(values[b].T),
            "wqk": wqk_host, "wv": wv_host, "bias": bias_host,
        })
    return in_maps


def assemble(results):
    out = np.zeros((B, S, DF), dtype=np.float32)
    for c in range(NCORES):
        b, h = c // 2, c % 2
        # [128, 2*8*64] p-major -> [2048, 64]
        arr = results[c]["out"].reshape(128, 2, IP // 128, DF)
        out[b, h * SQ:(h + 1) * SQ, :] = (
            arr.transpose(1, 2, 0, 3).reshape(SQ, DF))
    return out


def kernel(**inputs):
    nc = get_compiled()
    in_maps = make_in_maps(**inputs)
    res = run_bass_kernel_spmd(nc, in_maps, core_ids=list(range(NCORES)))
    return assemble(res.results)


# revision 27
# speedup vs baseline: 1.0083x; 1.0083x over previous
"""Distributed Trainium2 Bass kernel for a single attention head.

Reference computation (fp32 jax):
    q = queries @ Wq.T + bq        # [B,S,Df]
    k = keys    @ Wk.T + bk
    v = values  @ Wv.T + bv
    attn = softmax((q @ k.T) / sqrt(Df), axis=-1)
    out  = attn @ v                # [B,S,Df]

with B=4, S=4096, D_MODEL=1024, D_FEATURE=64.

Sharding: 8 cores = (batch b in 0..3) x (query-half h in 0..1).
Core c handles batch b=c//2, q rows [h*2048, (h+1)*2048). Each core gets
its q-half plus the FULL keys/values of its batch (no collectives).

v5 design notes (on top of v4, see git history / kernel_v4_baseline).
Machine constants: ACT exp on [128,1024] psum->sbuf costs ~1336ns
((~579+N)/1.2); 64 of them = 85.5us, the kernel floor. A 512-col
matmul issues every ~259ns warm; disjoint row/col-group matmuls run
concurrently. ~7us NEFF preamble before the first kernel op; each
dma_start costs ~730ns on its issuing queue.
  - DMA issues are spread across engines: sync does wqk+q blocks,
    gpsimd does k/v blocks, vector does bias. xk is laid out in
    256-col sub-blocks so the first exp waits on only ~1MB
    (wqk + q block0 + k sub0); the first pair's exps are [128,512].
  - v-projection is column-tiled: two 512-col blocks project
    concurrently in the two column halves of the PE array (out
    partitions 0:64 / 64:128 via tile_position), halving its PE
    stream time. Pairs run at kb1/3/5 (blocks 1+2, 3+4, 5+6);
    blocks 0 and 7 project solo at kb0/kb6.
  - tail: the filler drain stops 4 thunks short (G4 jc30/31), so
    merge_A + finals_A run under the last B-half exps; then the last
    G4 matmuls, merge_B, finals_B. finals batch 8 transposes into one
    psum strip, one strided reciprocal, 8 muls.
"""

import numpy as np
import ml_dtypes

import concourse.bass as bass
import concourse.mybir as mybir
import concourse.tile as tile
from concourse import bacc
from concourse.bass_utils import run_bass_kernel_spmd
from concourse.masks import make_identity

B = 4
S = 4096
DM = 1024
DF = 64
NCORES = 8
SQ = S // 2          # local q rows per core
MC = DM // 128       # 8 contraction chunks
MCP = MC // 2        # 4 DoubleRow chunk-pairs
NI = 512             # moving-operand tile (one PSUM bank of fp32)
NI2 = 256            # xk sub-block width
JC = S // 128        # 32 key chunks
NBQ = SQ // NI       # 4 q column blocks
NBK = S // NI        # 8 k/v column blocks
IP = SQ // 2         # 1024: i-rows per attention half
QW = MCP * 2 * NI    # 4096 fp8 elements per q block
QW2 = MCP * 2 * NI2  # 2048 fp8 elements per k sub-block
BF16 = mybir.dt.bfloat16
F8 = mybir.dt.float8e4
F32 = mybir.dt.float32
NP_BF16 = ml_dtypes.bfloat16
NP_F8 = ml_dtypes.float8_e4m3
EXP = mybir.ActivationFunctionType.Exp

# parked exp outputs: all 32 B-half chunks + A-half chunks 16..23
NPARK = JC + 8

# debug hook: when non-empty, build_kernel emits extra dram taps
DBG = {}

# vT2 layout: per 512-col block -> (partition base, column offset).
# Paired blocks share a column range, one in each partition half.
VMAP = {0: (0, 0), 1: (64, 0), 2: (0, 512), 3: (64, 512),
        4: (0, 1024), 5: (64, 1024), 6: (0, 1536), 7: (64, 1536)}


def _park_idx(jc, ipass):
    if ipass == 1:
        return jc
    assert 16 <= jc < 24
    return JC + (jc - 16)


def build_kernel(tc):
    nc = tc.nc
    xq = nc.dram_tensor("xq", [128, NBQ * QW], F8, kind="ExternalInput")
    xk = nc.dram_tensor("xk", [128, 2 * NBK * QW2], F8, kind="ExternalInput")
    xv = nc.dram_tensor("xv", [128, NBK * MC * NI], BF16, kind="ExternalInput")
    wqk = nc.dram_tensor("wqk", [128, 2 * MCP * 2 * 128], F8, kind="ExternalInput")
    wv = nc.dram_tensor("wv", [128, MC * DF], BF16, kind="ExternalInput")
    bias = nc.dram_tensor("bias", [128, 3], F32, kind="ExternalInput")
    out = nc.dram_tensor("out", [128, 2 * (IP // 128) * DF], F32,
                         kind="ExternalOutput")

    from contextlib import ExitStack

    with ExitStack() as ctx:
        const_pool = ctx.enter_context(tc.tile_pool(name="const", bufs=1))
        xq_pool = ctx.enter_context(tc.tile_pool(name="xq", bufs=4))
        xk_pool = ctx.enter_context(tc.tile_pool(name="xk", bufs=4))
        xv_pool = ctx.enter_context(tc.tile_pool(name="xv", bufs=4))
        act_pool = ctx.enter_context(tc.tile_pool(name="act", bufs=1))
        pt_pool = ctx.enter_context(tc.tile_pool(name="pt", bufs=8))
        outT_pool = ctx.enter_context(tc.tile_pool(name="outT", bufs=2))
        fin_pool = ctx.enter_context(tc.tile_pool(name="fin", bufs=2))
        # PSUM budget (8 banks): ppsum 2x[128,512] = 2 banks (proj psum
        # + vtrans scratch, then the G4 accumulator pair), spsum
        # 2x[128,1024] = 4 banks (scores double-buffer, then finals_B
        # scratch), opsum [65,1024] = 2 banks (G1/G2/G3 in sequence,
        # then finals_A scratch).
        ppsum = ctx.enter_context(tc.tile_pool(name="ppsum", bufs=2, space="PSUM"))
        spsum = ctx.enter_context(tc.tile_pool(name="spsum", bufs=2, space="PSUM"))
        opsum = ctx.enter_context(tc.tile_pool(name="opsum", bufs=1, space="PSUM"))

        # ---- DMAs, all on sync: only sync/scalar (hwdge engines)
        # stripe a dma_start across the 16 hardware queues; gpsimd
        # issues run on a single queue at ~1/10 the bandwidth. Issue
        # order = deadline order; the first exp needs only
        # wqk + q block0 + k sub-block0 (~1MB). bias rides scalar
        # (tiny, and scalar is idle before the first exp). ----
        wqk_sb = const_pool.tile([128, 2 * MCP * 2 * 128], F8, tag="wqk")
        nc.sync.dma_start(wqk_sb[:], wqk[:])
        bias_sb = const_pool.tile([128, 3], F32, tag="bias")
        nc.scalar.dma_start(bias_sb[:], bias[:])
        wv_sb = const_pool.tile([128, MC * DF], BF16, tag="wv")
        q_tiles = [None] * NBQ

        def load_q(i):
            t = xq_pool.tile([128, QW], F8, tag="xq")
            nc.sync.dma_start(t[:], xq[:, i * QW:(i + 1) * QW])
            q_tiles[i] = t

        k_tiles = {}
        v_tiles = {}

        def load_k0():
            # first k block in two sub-block dma_starts so kproj can
            # start after 256KB
            t = xk_pool.tile([128, 2 * QW2], F8, tag="xk")
            nc.sync.dma_start(t[:, 0:QW2], xk[:, 0:QW2])
            nc.sync.dma_start(t[:, QW2:2 * QW2], xk[:, QW2:2 * QW2])
            k_tiles[0] = t

        def load_k(i):
            t = xk_pool.tile([128, 2 * QW2], F8, tag="xk")
            nc.sync.dma_start(t[:], xk[:, i * 2 * QW2:(i + 1) * 2 * QW2])
            k_tiles[i] = t

        def load_v(i):
            t = xv_pool.tile([128, MC * NI], BF16, tag="xv")
            nc.sync.dma_start(t[:], xv[:, i * MC * NI:(i + 1) * MC * NI])
            v_tiles[i] = t

        load_q(0)
        load_k0()
        load_q(1)
        load_q(2)
        load_q(3)
        load_v(0)
        load_k(1)
        load_v(1)
        nc.sync.dma_start(wv_sb[:], wv[:])
        load_k(2)
        load_v(2)
        load_k(3)
        load_v(3)
        load_k(4)
        load_v(4)
        load_k(5)
        load_v(5)
        load_k(6)
        load_v(6)
        load_k(7)
        load_v(7)

        def q_ap(i):
            return q_tiles[i][:]

        # ---- constants computed on-chip ----
        scratch = const_pool.tile([DF, 1], F32, tag="scratch")
        ident = const_pool.tile([128, 128], BF16, tag="ident")
        make_identity(nc, ident[:])
        identf = const_pool.tile([128, 128], F32, tag="identf")
        make_identity(nc, identf[:])
        # preload the ACT exp table while DMAs stream
        nc.scalar.activation(scratch[:], identf[0:DF, 0:1], EXP)

        # ---- PE warm-up: dummy matmuls cover the HAM ramp until the
        # first q-projection's input DMA lands ----
        warm = opsum.tile([DF, 128], F32, tag="po")
        for _ in range(50):
            nc.tensor.matmul(warm[:], ident[:, 0:DF], ident[:], start=True, stop=True)

        # ---- persistent activations ----
        qT_sb = act_pool.tile([128, SQ], BF16, tag="qT")
        kT_sb = act_pool.tile([128, S], BF16, tag="kT")
        vT2_sb = act_pool.tile([128, 4 * NI], BF16, tag="vT2")
        v_sb = act_pool.tile([128, JC * (DF + 1)], BF16, tag="v")  # [128, 32*65]
        nc.gpsimd.memset(v_sb[:], 1.0)  # col DF of every block stays 1.0
        park_sb = act_pool.tile([128, NPARK * IP], BF16, tag="park")  # 10 MB

        def project_block_q(x_ap, i):
            """One 512-col q projection block: 4 fp8 DoubleRow matmuls
            with [w|w]-duplicated stationaries so the result lands in
            both partition halves (score pairing reads them as
            independent 64-row PE tiles)."""
            ps = ppsum.tile([128, NI], F32, tag="ps")
            for p in range(MCP):
                o = p * 256
                nc.tensor.matmul(
                    ps[:],
                    wqk_sb[:, o:o + 256].rearrange("p (s c) -> p s c", s=2),
                    x_ap[:, p * 2 * NI:(p + 1) * 2 * NI].rearrange(
                        "p (s c) -> p s c", s=2),
                    start=(p == 0), stop=(p == MCP - 1),
                    perf_mode=mybir.MatmulPerfMode.DoubleRow,
                )
            nc.vector.tensor_scalar_add(
                qT_sb[:, i * NI:(i + 1) * NI], ps[:],
                bias_sb[0:128, 0:1])

        def project_block_k(i):
            """One 512-col k projection block as two 256-col halves so
            the first scores can start after half the block's DMA."""
            t = k_tiles[i]
            ps = ppsum.tile([128, NI], F32, tag="ps")
            for sub in range(2):
                for p in range(MCP):
                    o = MCP * 2 * 128 + p * 256
                    nc.tensor.matmul(
                        ps[:, sub * NI2:(sub + 1) * NI2],
                        wqk_sb[:, o:o + 256].rearrange("p (s c) -> p s c", s=2),
                        t[:, sub * QW2 + p * 2 * NI2:
                          sub * QW2 + (p + 1) * 2 * NI2].rearrange(
                            "p (s c) -> p s c", s=2),
                        start=(p == 0), stop=(p == MCP - 1),
                        perf_mode=mybir.MatmulPerfMode.DoubleRow,
                    )
                nc.vector.tensor_scalar_add(
                    kT_sb[:, i * NI + sub * NI2:i * NI + (sub + 1) * NI2],
                    ps[:, sub * NI2:(sub + 1) * NI2],
                    bias_sb[0:128, 1:2])

        def project_v_pair(bA, bB, mcs, evict):
            """Part of a column-tiled v projection block pair: block bA
            lands in psum partitions 0:64 (array col groups 0-1), bB in
            64:128 (groups 2-3); the two matmuls of each m-chunk run
            concurrently. Separate banks so the two accumulation
            groups don't share a psum zero region."""
            pbA, coA = VMAP[bA]
            pbB, coB = VMAP[bB]
            assert pbA == 0 and pbB == 64 and coA == coB
            if 0 in mcs:
                project_v_pair.psA = ppsum.tile([DF, NI], F32, tag="ps")
                project_v_pair.psB = ppsum.tile([128, NI], F32, tag="ps")
            psA = project_v_pair.psA
            psB = project_v_pair.psB
            for mc_i in mcs:
                w_ap = wv_sb[:, mc_i * DF:(mc_i + 1) * DF]
                nc.tensor.matmul(
                    psA[:], w_ap,
                    v_tiles[bA][:, mc_i * NI:(mc_i + 1) * NI],
                    start=(mc_i == 0), stop=(mc_i == MC - 1),
                    tile_position=(0, 0),
                )
                nc.tensor.matmul(
                    psB[DF:128, :], w_ap,
                    v_tiles[bB][:, mc_i * NI:(mc_i + 1) * NI],
                    start=(mc_i == 0), stop=(mc_i == MC - 1),
                    tile_position=(0, 64),
                )
            if evict:
                nc.vector.tensor_scalar_add(
                    vT2_sb[0:DF, coA:coA + NI], psA[:], bias_sb[0:DF, 2:3])
                nc.vector.tensor_scalar_add(
                    vT2_sb[DF:128, coA:coA + NI], psB[DF:128, :],
                    bias_sb[DF:128, 2:3])

        def vtrans(jc):
            b = jc // 4
            pb, co = VMAP[b]
            pv = ppsum.tile([128, DF], BF16, tag="ps")
            src = vT2_sb[pb:pb + DF, co + (jc % 4) * 128:co + (jc % 4) * 128 + 128]
            nc.tensor.transpose(pv[:], src, ident[pb:pb + DF, pb:pb + DF])
            nc.vector.tensor_copy(
                v_sb[:, jc * (DF + 1):jc * (DF + 1) + DF], pv[:])

        def v_slice(jc):
            return v_sb[:, jc * (DF + 1):jc * (DF + 1) + DF + 1]

        # ---- thunk queue: every attn@v matmul (and accumulator
        # rotation step) is drip-fed between exp groups. push_buf lags
        # exp-adjacent thunks one pair so popped matmuls never stall
        # the PE FIFO on a not-yet-run exp. ----
        filler_q = []
        push_buf = []

        def pop_fillers(n=8):
            for _ in range(min(n, len(filler_q))):
                filler_q.pop(0)()

        poG = {}
        outT_A = outT_pool.tile([DF + 1, IP], F32, tag="outT", name="outT_A")
        outT_B = outT_pool.tile([DF + 1, IP], F32, tag="outT", name="outT_B")

        def attnv_mm(gen, jc, src_ap, ii, first, last):
            po = poG[gen]
            if gen == "G4":
                nc.tensor.matmul(
                    po[ii][:], v_slice(jc), src_ap[:, ii * NI:(ii + 1) * NI],
                    start=first, stop=last)
            else:
                nc.tensor.matmul(
                    po[:, ii * NI:(ii + 1) * NI], v_slice(jc),
                    src_ap[:, ii * NI:(ii + 1) * NI],
                    start=first, stop=last)

        def push_mms(buf, gen, jc, src, first, last):
            for ii in range(2):
                buf.append(lambda gen=gen, jc=jc, src=src, ii=ii:
                           attnv_mm(gen, jc, src, ii, first, last))

        def park_ap(jc, ipass):
            idx = _park_idx(jc, ipass)
            return park_sb[:, idx * IP:(idx + 1) * IP]

        def hooks(jc, ipass, pts):
            """Queue the attn@v matmuls for one exp'd chunk."""
            if ipass == 1:
                if jc >= 28:
                    push_mms(push_buf, "G4", jc, pts, False, jc == 31)
                return
            if jc < 16:
                push_mms(push_buf, "G1", jc, pts, jc == 0, jc == 15)
            elif jc >= 24:
                push_mms(push_buf, "G3", jc, pts, False, jc == 31)

        def attn_pair(jc0, ipasses=(0, 1), mid=None, pop_n=8):
            """Scores + exp for the requested i-halves of chunks jc0,
            jc0+1. Per half: scores, exps, `mid` (once), fillers; hook
            thunks lag one attn_pair via push_buf."""
            filler_q.extend(push_buf)
            push_buf.clear()
            pending = None
            for ipass in ipasses:
                io = ipass * IP
                ss0 = spsum.tile([128, IP], F32, tag="ss", name="ss0")
                ss1 = spsum.tile([128, IP], F32, tag="ss", name="ss1")
                sss = [ss0, ss1]
                for ii in range(IP // NI):
                    for t in range(2):
                        jc = jc0 + t
                        p0 = t * DF
                        nc.tensor.matmul(
                            sss[t][:, ii * NI:(ii + 1) * NI],
                            kT_sb[p0:p0 + DF, jc * 128:(jc + 1) * 128],
                            qT_sb[p0:p0 + DF, io + ii * NI:io + (ii + 1) * NI],
                            start=True, stop=True,
                        )
                if pending:
                    for args in pending:
                        hooks(*args)
                    pending = None
                group = []
                for t in range(2):
                    jc = jc0 + t
                    parked = (ipass == 1) or (16 <= jc < 24)
                    if parked:
                        pts = park_ap(jc, ipass)
                    else:
                        pts = pt_pool.tile([128, IP], BF16, tag="pt")
                    nc.scalar.activation(pts[:], sss[t][:], EXP, scale=0.125)
                    group.append((jc, ipass, pts))
                if mid is not None:
                    mid()
                    mid = None
                pop_fillers(pop_n)
                pending = group
            for args in pending:
                hooks(*args)

        # ================= kb0: get ACT started ASAP =================
        # pair(0) is hand-rolled with [128,512] exps so the first exp
        # waits only on wqk + q block0 + k sub-block0 (~1MB of DMA).
        poG["G1"] = opsum.tile([DF + 1, IP], F32, tag="po", name="poG1")
        project_block_q(q_ap(0), 0)
        project_block_k(0)
        ss0 = spsum.tile([128, IP], F32, tag="ss", name="ss0")
        ss1 = spsum.tile([128, IP], F32, tag="ss", name="ss1")
        pts0 = pt_pool.tile([128, IP], BF16, tag="pt")
        pts1 = pt_pool.tile([128, IP], BF16, tag="pt")
        for ii in range(2):
            for t in range(2):
                ss = [ss0, ss1][t]
                pts = [pts0, pts1][t]
                p0 = t * DF
                nc.tensor.matmul(
                    ss[:, ii * NI:(ii + 1) * NI],
                    kT_sb[p0:p0 + DF, t * 128:(t + 1) * 128],
                    qT_sb[p0:p0 + DF, ii * NI:(ii + 1) * NI],
                    start=True, stop=True,
                )
                nc.scalar.activation(pts[:, ii * NI:(ii + 1) * NI],
                                     ss[:, ii * NI:(ii + 1) * NI],
                                     EXP, scale=0.125)
            if ii == 0:
                project_block_q(q_ap(1), 1)
        hooks(0, 0, pts0)
        hooks(1, 0, pts1)
        # q blocks 2,3 feed the deferred B halves starting at def(0)
        project_block_q(q_ap(2), 2)
        project_block_q(q_ap(3), 3)

        def kb0_mid():
            project_block_k(1)
            project_v_pair(0, 1, range(MC), evict=True)
            for jc in range(4):
                vtrans(jc)

        # B halves of chunks 0..15 are deferred filler ACT work: two
        # deferred pairs per kb at kb0..kb3 give every kb exactly 8
        # exp tiles, and their emission before each kb's projection
        # mountain keeps ACT fed while PE grinds. pair(2)'s pops are
        # held (pop_n=0) so the G1 jc0/1 matmuls queue after
        # kb0_mid's vtrans.
        deferred_b = [0, 2, 4, 6, 8, 10, 12, 14]
        attn_pair(2, ipasses=(0,), pop_n=0)
        attn_pair(deferred_b.pop(0), ipasses=(1,), mid=kb0_mid)
        attn_pair(deferred_b.pop(0), ipasses=(1,))

        # v-projection plan: column-tiled pairs, scheduled into kbs
        # with ACT slack ((0,1) in kb0's mid); vtrans for chunks 16+
        # only needs to precede the G3/G4 park pops at kb6/7, so
        # blocks 4..7 project late and their vtrans ride kb5-7.
        VPAIR = {2: (2, 3), 5: (4, 5), 6: (6, 7)}
        VT_MID1 = {1: [4, 5], 2: [], 3: [12, 13], 4: [], 5: [],
                   6: [20, 21, 22, 23], 7: [26, 27, 28, 29, 30, 31]}
        VT_MID2 = {1: [6, 7], 2: [8, 9, 10, 11], 3: [14, 15], 4: [],
                   5: [16, 17, 18, 19], 6: [24, 25], 7: []}

        # ================= kb loop 1..7 =================
        for kb in range(1, NBK):
            # flush lagged thunks so kb-head rotation steps queue after
            # the previous pair's attn@v matmuls
            filler_q.extend(push_buf)
            push_buf.clear()

            def mid1(kb=kb):
                # vtrans first: the vpair psum pair must be the last
                # ppsum allocations before mid2's eviction
                for jc in VT_MID1[kb]:
                    vtrans(jc)
                if kb in VPAIR:
                    project_v_pair(*VPAIR[kb], mcs=range(4), evict=False)

            def mid2(kb=kb):
                if kb in VPAIR:
                    project_v_pair(*VPAIR[kb], mcs=range(4, MC), evict=True)
                if kb < 7:
                    project_block_k(kb + 1)
                for jc in VT_MID2[kb]:
                    vtrans(jc)

            pop_n = 8

            if kb == 4:
                # rotate the opsum slot G1 -> G2 in queue order, after
                # G1's lagged jc15 matmuls
                def rot_g2():
                    nc.vector.tensor_copy(outT_A[:], poG["G1"][:])
                    if DBG.get("taps"):
                        g1tap = nc.dram_tensor("tap_g1", [DF + 1, IP], F32,
                                               kind="ExternalOutput")
                        nc.sync.dma_start(g1tap[:], outT_A[:])
                    poG["G2"] = opsum.tile([DF + 1, IP], F32, tag="po",
                                           name="poG2")
                filler_q.append(rot_g2)
                for jc in range(0, 16):
                    push_mms(filler_q, "G2", jc, park_ap(jc, 1),
                             jc == 0, jc == 15)
            if kb == 6:
                def rot_g3():
                    nc.vector.tensor_copy(outT_B[:], poG["G2"][:])
                    poG["G3"] = opsum.tile([DF + 1, IP], F32, tag="po",
                                           name="poG3")
                filler_q.append(rot_g3)
                for jc in range(16, 24):
                    push_mms(filler_q, "G3", jc, park_ap(jc, 0),
                             jc == 16, False)
            if kb == 7:
                # after kb6's last vtrans, the ppsum banks become the
                # two G4 accumulator halves
                def alloc_g4():
                    g4a = ppsum.tile([DF + 1, NI], F32, tag="ps", name="g4a")
                    g4b = ppsum.tile([DF + 1, NI], F32, tag="ps", name="g4b")
                    poG["G4"] = [g4a, g4b]
                filler_q.append(alloc_g4)
                for jc in range(16, 28):
                    push_mms(filler_q, "G4", jc, park_ap(jc, 1),
                             jc == 16, False)

            if kb in (1, 2, 3):
                attn_pair(deferred_b.pop(0), ipasses=(1,), pop_n=pop_n)
                attn_pair(deferred_b.pop(0), ipasses=(1,), pop_n=pop_n)
            ip = (0,) if kb <= 3 else (0, 1)
            attn_pair(4 * kb, ipasses=ip, mid=mid1, pop_n=pop_n)
            attn_pair(4 * kb + 2, ipasses=ip, mid=mid2, pop_n=pop_n)

        assert not deferred_b

        # ---- finals: 8 transposes into one psum strip, one strided
        # reciprocal over the denominator columns, 8 muls, one DMA ----
        def finals(outT_sb, ipass, pool, tag):
            # 8 transposes into one 2-bank psum strip (128-col slots so
            # no [128,65] output crosses a bank), one strided
            # reciprocal over the 8 denominator columns, 8 muls.
            ob = fin_pool.tile([128, (IP // 128) * DF], F32, tag="ob")
            pf = pool.tile([128, IP], F32, tag=tag, name=f"pf{ipass}")
            for c in range(IP // 128):
                nc.tensor.transpose(
                    pf[:, c * 128:c * 128 + DF + 1],
                    outT_sb[:, c * 128:(c + 1) * 128],
                    identf[0:DF + 1, 0:DF + 1])
            rcp = fin_pool.tile([128, IP // 128], F32, tag="rcp")
            nc.vector.reciprocal(
                rcp[:], pf[:].rearrange("p (c k) -> p c k", k=128)[:, :, DF:DF + 1])
            nc.vector.tensor_tensor(
                ob[:].rearrange("p (c k) -> p c k", k=DF),
                pf[:].rearrange("p (c k) -> p c k", k=128)[:, :, 0:DF],
                rcp[:].rearrange("p (c k) -> p c k", k=1).broadcast_to(
                    [128, IP // 128, DF]),
                op=mybir.AluOpType.mult)
            half = (IP // 128) * DF
            nc.sync.dma_start(out[:, ipass * half:(ipass + 1) * half], ob[:])

        # ---- tail: drain all thunks except the last G4 pair (jc30/31,
        # gated on the final B exps), so merge_A + finals_A overlap the
        # last exps; then the G4 tail, merge_B, finals_B. ----
        filler_q.extend(push_buf)
        push_buf.clear()
        assert len(filler_q) >= 4
        while len(filler_q) > 4:
            filler_q.pop(0)()
        if DBG.get("taps"):
            g3tap = nc.dram_tensor("tap_g3", [DF + 1, IP], F32,
                                   kind="ExternalOutput")
            fin3 = fin_pool.tile([DF + 1, IP], F32, tag="ob", name="fin3")
            nc.vector.tensor_copy(fin3[:], poG["G3"][:])
            nc.sync.dma_start(g3tap[:], fin3[:])
        nc.vector.tensor_add(outT_A[:], outT_A[:], poG["G3"][:])
        finals(outT_A, 0, opsum, "po")
        while filler_q:
            filler_q.pop(0)()
        nc.vector.tensor_add(outT_B[:, 0:NI], outT_B[:, 0:NI], poG["G4"][0][:])
        nc.vector.tensor_add(outT_B[:, NI:IP], outT_B[:, NI:IP], poG["G4"][1][:])
        finals(outT_B, 1, spsum, "ss")


_COMPILED = None


def get_compiled():
    global _COMPILED
    if _COMPILED is None:
        nc = bacc.Bacc("TRN2", target_bir_lowering=False, debug=False,
                       enable_asserts=False, num_devices=NCORES)
        with tile.TileContext(nc) as tc:
            build_kernel(tc)
        nc.compile()
        _COMPILED = nc
    return _COMPILED


def _to_pair_major(xT, ni):
    """[DM, s_len] fp32 -> fp8 [128, nblk * 4 pairs * 2 slots * ni]."""
    s_len = xT.shape[1]
    nblk = s_len // ni
    r = xT.reshape(MCP, 2, 128, nblk, ni).transpose(2, 3, 0, 1, 4)
    return np.ascontiguousarray(r.reshape(128, nblk * MCP * 2 * ni)).astype(NP_F8)


def _w_pair_major(W):
    """W [64, DM] fp32 -> fp8 [128, 4 pairs * 2 slots * 128] with [w|w] dup."""
    WT = np.ascontiguousarray(np.asarray(W, np.float32).T)   # [DM, 64]
    dup = np.concatenate([WT, WT], axis=1)                   # [DM, 128]
    r = dup.reshape(MCP, 2, 128, 128).transpose(2, 0, 1, 3)
    return np.ascontiguousarray(r.reshape(128, MCP * 2 * 128)).astype(NP_F8)


def _to_block_major(xT):
    """[DM, s_len] -> bf16 [128, nblk*MC*NI]: 512-col blocks, m-chunk-major."""
    s_len = xT.shape[1]
    nblk = s_len // NI
    return np.ascontiguousarray(
        xT.reshape(MC, 128, nblk, NI).transpose(1, 2, 0, 3)
        .reshape(128, nblk * MC * NI)).astype(NP_BF16)


def _w_chunk_major(W, dt):
    """W [64, DM] -> [128, MC*64]: per m-chunk [128, 64] stationaries."""
    WT = np.ascontiguousarray(np.asarray(W, np.float32).T)   # [DM, 64]
    return np.ascontiguousarray(
        WT.reshape(MC, 128, DF).transpose(1, 0, 2).reshape(128, MC * DF)
    ).astype(dt)


def make_in_maps(queries, keys, values, Wq, bq, Wk, bk, Wv, bv):
    queries = np.asarray(queries, dtype=np.float32)
    keys = np.asarray(keys, dtype=np.float32)
    values = np.asarray(values, dtype=np.float32)
    wqk_host = np.concatenate([_w_pair_major(Wq), _w_pair_major(Wk)], axis=1)
    wv_host = _w_chunk_major(Wv, NP_BF16)
    bias64 = np.stack(
        [np.asarray(bq), np.asarray(bk), np.asarray(bv)], axis=1
    ).astype(np.float32)
    bias_host = np.concatenate([bias64, bias64], axis=0)     # [128, 3]

    in_maps = []
    for c in range(NCORES):
        b, h = c // 2, c % 2
        in_maps.append({
            "xq": _to_pair_major(queries[b, h * SQ:(h + 1) * SQ, :].T, NI),
            "xk": _to_pair_major(keys[b].T, NI2),
            "xv": _to_block_major(values[b].T),
            "wqk": wqk_host, "wv": wv_host, "bias": bias_host,
        })
    return in_maps


def assemble(results):
    out = np.zeros((B, S, DF), dtype=np.float32)
    for c in range(NCORES):
        b, h = c // 2, c % 2
        # [128, 2*8*64] p-major -> [2048, 64]
        arr = results[c]["out"].reshape(128, 2, IP // 128, DF)
        out[b, h * SQ:(h + 1) * SQ, :] = (
            arr.transpose(1, 2, 0, 3).reshape(SQ, DF))
    return out


def kernel(**inputs):
    nc = get_compiled()
    in_maps = make_in_maps(**inputs)
    res = run_bass_kernel_spmd(nc, in_maps, core_ids=list(range(NCORES)))
    return assemble(res.results)
